# revision 20
# baseline (speedup 1.0000x reference)
"""GAT layer (gnn_message_passing) on 8 Trainium2 NeuronCores.

Strategy (dst-owner sharding, dst-major slot layout):
  - Node features z = h @ W.T are computed sharded in bf16: core c computes z
    for its 6250 "owned" destination nodes (zd = z@a_dst folded in as one
    extra matmul output column).
  - Each core builds 256B table rows (z in bf16[128]) and an AllGather
    replicates the full 50k-row table to every core.
  - Edges are bucketed by the owner of dst, so each core holds the COMPLETE
    in-edge set of its nodes: segment softmax is fully local.  Softmax uses
    the no-max-subtraction form (scores are O(10), exp stays in f32 range);
    alpha is never materialized: out = (sum_e ex_e * z[src_e]) / (sum_e ex_e).
  - Per core the owned nodes are sorted by in-degree and laid out dst-major:
    partition p = node slot, free column = edge rank.  dma_gather fetches
    z[src] rows straight into that layout.  zs[src] = z[src]@a_src is
    recomputed per edge from the gathered row by a DVE multiply+reduce
    (instead of shipping zs in the table); zd[dst] is a per-partition
    broadcast; the weighted segment-sum runs on the TensorEngine as a chain
    of identity-lhsT matmuls accumulating in PSUM.
  - All SWDGE ops (gathers + scatters) use prepare_only descriptors with
    explicit trigger_dma so the GpSimd engine only pays descriptor
    generation; DMA transfers overlap the next chunk's desc-gen.  Gather
    descriptors are generated during the AllGather window.
  - dma_gather indices are int16, so the gather is split into two calls over
    the low/high halves of the table (separate slot layouts per half, merged
    by dma_scatter_add into a node-ordered accumulator).
  - Zero-in-degree nodes get a fake self-edge on the host so out == z for
    them exactly (no select pass needed).
"""

import numpy as np
import ml_dtypes

import concourse.bass as bass
import concourse.mybir as mybir
import concourse.tile as tile
from concourse import bacc
from concourse import library_config
from concourse.bass import ts
from concourse.bass_utils import run_bass_kernel_spmd

F32 = mybir.dt.float32
BF16 = mybir.dt.bfloat16
I16 = mybir.dt.int16

NC = 8          # cores
P = 128         # partitions
IN_DIM = 256
OUT_DIM = 128
KCH = IN_DIM // P       # 2 k-chunks for the feature matmul
ROW_ELEMS = 128         # bf16 elems per table row (256 B)
ACC_STRIDE = 320        # f32 elems per accumulator row (1280 B)
ACC_OFF_H = 160         # second half's column offset within an acc row
SCAT_ELEMS = 129        # f32 elems scattered per slot ([agg128 | den])
ZDP_STRIDE = 64         # f32 stride of the zd permute buffer (256 B)
CHUNK_COLS = 32         # max gather columns per chunk
PREP_DEPTH = 4          # gather preps issued before the first trigger
USE_PREP = True         # prepare_only + trigger_dma pipelining
SCAT_Q = 0              # SWDGE queue for scatters (gathers use queue 0)


class Cfg:
    def __init__(self, n_nodes, n_edges):
        assert n_nodes % NC == 0
        self.N = n_nodes
        self.E = n_edges
        self.NPC = n_nodes // NC
        self.NPAD = ((self.NPC + P - 1) // P) * P
        self.NT = self.NPAD // P
        assert n_nodes % 2 == 0
        self.HALF = n_nodes // 2
        self.HALF_ROWS = (NC // 2) * self.NPAD  # table rows per half


def _wrap16(flat, dtype=np.int16):
    """flat[i] -> [128, len/16] with flat[i] at [i%16, i//16], replicated x8."""
    n = flat.shape[0]
    assert n % 16 == 0
    w = flat.reshape(n // 16, 16).T.astype(dtype)  # [16, n/16]
    return np.tile(w, (8, 1))


def host_prep(cfg, src, dst):
    """Build the common tile structure + per-core index/mask arrays."""
    N, NPC, NPAD, NT, HALF = cfg.N, cfg.NPC, cfg.NPAD, cfg.NT, cfg.HALF
    src = np.asarray(src, np.int64).copy()
    dst = np.asarray(dst, np.int64).copy()

    # fake self-edges for isolated (zero in-degree) nodes -> out == z exactly
    deg_tot = np.bincount(dst, minlength=N)
    iso = np.nonzero(deg_tot == 0)[0]
    if iso.size:
        src = np.concatenate([src, iso])
        dst = np.concatenate([dst, iso])

    owner = dst // NPC
    halves = (src >= HALF).astype(np.int64)

    # table row of a global node: owner block of NPAD rows + local index
    table_row = (src // NPC) * NPAD + (src % NPC)
    table_local = table_row - (halves * cfg.HALF_ROWS)
    assert table_local.max() < 32768 and table_local.min() >= 0

    per = {}
    deg_sorted_all = []
    for c in range(NC):
        for h in (0, 1):
            m = (owner == c) & (halves == h)
            es = table_local[m]               # gather index of each edge
            ed = dst[m] - c * NPC             # local dst node
            deg = np.bincount(ed, minlength=NPAD)
            order = np.argsort(-deg, kind="stable")  # node_of_slot [NPAD]
            sL = np.empty(NPAD, np.int64)
            sL[order] = np.arange(NPAD)              # slot of node
            per[(c, h)] = dict(es=es, ed=ed, deg=deg, order=order, sL=sL)
            deg_sorted_all.append(deg[order])

    # common tile widths
    W = np.zeros(NT, np.int64)
    for dsrt in deg_sorted_all:
        W = np.maximum(W, dsrt[::P][:NT])
    NTp = int(np.nonzero(W > 0)[0][-1]) + 1 if (W > 0).any() else 0
    W = W[:NTp]
    colstart = np.concatenate([[0], np.cumsum(W)]).astype(np.int64)
    C = int(colstart[-1])
    CP = C  # gather cols total per half

    # chunks: runs of equal W, capped at CHUNK_COLS columns
    chunks = []  # (t0, nt, W)
    t = 0
    while t < NTp:
        w = int(W[t])
        nt = 1
        while (
            t + nt < NTp
            and int(W[t + nt]) == w
            and (nt + 1) * w <= CHUNK_COLS
        ):
            nt += 1
        chunks.append((t, nt, w))
        t += nt

    # per-(core,half) flat arrays
    data = {}
    for c in range(NC):
        for h in (0, 1):
            d = per[(c, h)]
            es, ed, deg, order, sL = (
                d["es"], d["ed"], d["deg"], d["order"], d["sL"],
            )
            slot = sL[ed]
            tile_of = slot // P
            part_of = slot % P
            # rank of each edge within its dst node
            o = np.argsort(slot, kind="stable")
            slot_s = slot[o]
            es_s = es[o]
            tile_s = tile_of[o]
            part_s = part_of[o]
            counts = np.bincount(slot_s, minlength=NPAD)
            starts = np.concatenate([[0], np.cumsum(counts)])[:-1]
            rank = np.arange(slot_s.size) - starts[slot_s]
            keep = tile_s < NTp
            assert keep.all(), "edge landed outside processed tiles"
            cglob = colstart[tile_s] + rank
            assert (rank < W[tile_s]).all()
            pos = cglob * P + part_s

            flat_idx = np.zeros(CP * P, np.int16)
            flat_idx[pos] = es_s.astype(np.int16)
            mask = np.zeros((P, CP), np.float32)
            mask[part_s, cglob] = 1.0

            data[(c, h)] = dict(
                gidx=_wrap16(flat_idx),
                gmask=mask,
                mscat=_wrap16(order[: NTp * P].astype(np.int16)),
                zdscat=_wrap16(sL.astype(np.int16)),
            )

    struct = dict(W=W, NTp=NTp, colstart=colstart, C=C, chunks=chunks)
    return struct, data


def build_program(cfg, struct):
    NPAD, NT, NTp, C = cfg.NPAD, cfg.NT, struct["NTp"], struct["C"]
    W, colstart, chunks = struct["W"], struct["colstart"], struct["chunks"]

    nc = bacc.Bacc(
        "TRN2", target_bir_lowering=False, debug=False, num_devices=NC,
        num_swdge_queues=1 + (SCAT_Q > 0),
        dynamic_dma_scratch_size=32768,
    )

    # I/O
    hT = nc.dram_tensor("hT", [IN_DIM, NPAD], BF16, kind="ExternalInput").ap()
    W_aug = nc.dram_tensor("W_aug", [IN_DIM, 129], BF16, kind="ExternalInput").ap()
    ident_in = nc.dram_tensor("ident", [P, P], BF16, kind="ExternalInput").ap()
    asrc_in = nc.dram_tensor("asrc", [P, OUT_DIM], BF16, kind="ExternalInput").ap()
    gidx_in = nc.dram_tensor("gidx", [2, P, C * 8], I16, kind="ExternalInput").ap()
    gmask_in = nc.dram_tensor("gmask", [2, P, C], F32, kind="ExternalInput").ap()
    mscat_in = nc.dram_tensor("mscat", [2, P, NTp * 8], I16, kind="ExternalInput").ap()
    zdscat_in = nc.dram_tensor("zdscat", [2, P, NPAD // 16], I16, kind="ExternalInput").ap()

    out = nc.dram_tensor("out", [NPAD, OUT_DIM], F32, kind="ExternalOutput").ap()
    acc = nc.dram_tensor("acc", [NPAD, ACC_STRIDE], F32, kind="ExternalOutput").ap()
    zdperm = nc.dram_tensor("zdperm", [2 * NPAD, ZDP_STRIDE], F32, kind="ExternalOutput").ap()

    table_own = nc.dram_tensor("table_own", [NPAD, ROW_ELEMS], BF16, kind="Internal").ap()
    table = nc.dram_tensor(
        "table", [NC * NPAD, ROW_ELEMS], BF16, kind="Internal", addr_space="Shared"
    ).ap()

    nc.gpsimd.load_library(library_config.mlp)

    gsems = [nc.alloc_semaphore(f"gdma{i}") for i in range(8)]
    ssem = nc.alloc_semaphore("scat_dma")

    with tile.TileContext(nc) as tc:
        with tc.tile_pool(name="const", bufs=1) as constp:
            ident = constp.tile([P, P], BF16)
            nc.sync.dma_start(ident, ident_in)
            asrc = constp.tile([P, OUT_DIM], BF16)
            nc.sync.dma_start(asrc, asrc_in)

            # index tiles needed by SWDGE preps (metadata read at prep time)
            zdsc_sb = []
            for h in (0, 1):
                zdsc = constp.tile([P, NPAD // 16], I16, tag=f"zdsc{h}")
                nc.sync.dma_start(zdsc, zdscat_in[h])
                zdsc_sb.append(zdsc)

            # ---------------- phase 1: z / zd + table ----------------
            with (
                tc.tile_pool(name="ph1c", bufs=1) as ph1c,
                tc.tile_pool(name="ph1", bufs=2) as ph1,
                tc.tile_pool(name="ph1ps", bufs=2, space="PSUM") as ph1ps,
            ):
                wsb = ph1c.tile([P, KCH, 129], BF16)
                nc.sync.dma_start(wsb, W_aug.rearrange("(ko ki) m -> ki ko m", ki=P))

                hsb = ph1c.tile([P, KCH, NPAD], BF16)
                nc.sync.dma_start(hsb, hT.rearrange("(ko ki) n -> ki ko n", ki=P))

                zd_sb = ph1c.tile([P, NT, 1], F32)

                for t in range(NT):
                    ps = ph1ps.tile([P, 129], F32)
                    for k in range(KCH):
                        nc.tensor.matmul(
                            ps,
                            lhsT=hsb[:, k, ts(t, P)],
                            rhs=wsb[:, k, :],
                            start=(k == 0),
                            stop=(k == KCH - 1),
                        )
                    row = ph1.tile([P, ROW_ELEMS], BF16, tag="rowbf")
                    nc.scalar.copy(row, ps[:, 0:OUT_DIM])
                    nc.vector.tensor_copy(zd_sb[:, t, :], ps[:, 128:129])
                    nc.sync.dma_start(table_own[ts(t, P), :], row)

                nc.gpsimd.collective_compute(
                    "AllGather",
                    mybir.AluOpType.bypass,
                    replica_groups=[list(range(NC))],
                    ins=[table_own],
                    outs=[table],
                )
                # zd permute: plain scatters, hidden under the collective
                for h in (0, 1):
                    nc.gpsimd.dma_scatter_add(
                        out_ap=zdperm[h * NPAD:(h + 1) * NPAD, 0:1],
                        in_ap=zd_sb,
                        idxs_ap=zdsc_sb[h],
                        num_idxs=NPAD,
                        num_idxs_reg=NPAD,
                        elem_size=1,
                        elem_step=ZDP_STRIDE,
                        single_packet=NPAD <= 1024,
                    )

            # ---------------- phase 2: edges ----------------
            with (
                tc.tile_pool(name="meta", bufs=1) as metap,
                tc.tile_pool(name="gbuf", bufs=PREP_DEPTH + 2) as gbuf,
                tc.tile_pool(name="ebuf", bufs=3) as ebuf,
                tc.tile_pool(name="aggb", bufs=1) as aggb,
                tc.tile_pool(name="ps2", bufs=6, space="PSUM") as ps2,
            ):
                zdslots = []
                gidx_sb = []
                gmask_sb = []
                mscat_sb = []
                for h in (0, 1):
                    z = metap.tile([P, NT, 1], F32, tag=f"zds{h}")
                    nc.sync.dma_start(
                        z,
                        zdperm[h * NPAD:(h + 1) * NPAD, :]
                        .rearrange("(t p) d -> p t d", p=P)[:, :, 0:1],
                    )
                    zdslots.append(z)
                    g = metap.tile([P, C * 8], I16, tag=f"gidx{h}")
                    nc.sync.dma_start(g, gidx_in[h])
                    gidx_sb.append(g)
                    m = metap.tile([P, C], F32, tag=f"gmask{h}")
                    nc.sync.dma_start(m, gmask_in[h])
                    gmask_sb.append(m)
                    s = metap.tile([P, NTp * 8], I16, tag=f"mscat{h}")
                    nc.sync.dma_start(s, mscat_in[h])
                    mscat_sb.append(s)

                agg0 = aggb.tile([P, NTp, SCAT_ELEMS], F32, tag="agg0")
                agg1 = aggb.tile([P, NTp, SCAT_ELEMS], F32, tag="agg1")
                agg = {0: agg0, 1: agg1}

                # flat chunk list across both halves
                allch = [(h, t0, ntc, w) for h in (0, 1) for (t0, ntc, w) in chunks]
                nch = len(allch)
                gtiles = [None] * nch
                acc0_prep_at = min(len(chunks) + len(chunks) // 2, nch - 1)

                def emit_prep(k):
                    h, t0, ntc, w = allch[k]
                    cc = ntc * w
                    c0 = int(colstart[t0])
                    G = gbuf.tile([P, CHUNK_COLS, ROW_ELEMS], BF16, tag="G")
                    kwargs = (
                        dict(prepare_only=True, sem=gsems[k % 8], queue_num=0)
                        if USE_PREP else {}
                    )
                    nc.gpsimd.dma_gather(
                        out_ap=G[:, 0:cc, :],
                        in_ap=table[h * cfg.HALF_ROWS:(h + 1) * cfg.HALF_ROWS, :],
                        idxs_ap=gidx_sb[h][:, c0 * 8:(c0 + cc) * 8],
                        num_idxs=cc * P,
                        num_idxs_reg=cc * P,
                        elem_size=ROW_ELEMS,
                        single_packet=cc * P <= 1024,
                        **kwargs,
                    )
                    gtiles[k] = G

                def emit_acc_scatter_prep(h):
                    kwargs = (
                        dict(prepare_only=True, sem=ssem, queue_num=SCAT_Q)
                        if USE_PREP else {}
                    )
                    nc.gpsimd.dma_scatter_add(
                        out_ap=acc[:, (ACC_OFF_H if h else 0):(ACC_OFF_H if h else 0) + SCAT_ELEMS],
                        in_ap=agg[h],
                        idxs_ap=mscat_sb[h],
                        num_idxs=NTp * P,
                        num_idxs_reg=NTp * P,
                        elem_size=SCAT_ELEMS,
                        elem_step=ACC_STRIDE,
                        single_packet=NTp * P <= 1024,
                        **kwargs,
                    )

                def emit_consumer(k):
                    h, t0, ntc, w = allch[k]
                    cc = ntc * w
                    c0 = int(colstart[t0])
                    G = gtiles[k]
                    if USE_PREP:
                        # Tile's DMASW signal for prepped gathers fires at
                        # descriptor-arm time, not DMA completion; gate on the
                        # descriptor-baked per-chunk sem instead.  Rotation 8
                        # exceeds the G-buffer WAR window, and per-queue ring
                        # FIFO order makes the cumulative target exact.
                        nc.vector.wait_ge(gsems[k % 8], 16 * (k // 8 + 1))
                    # zs[src] per edge: dot(gathered z row, a_src)
                    zsp = ebuf.tile([P, CHUNK_COLS, OUT_DIM], BF16, tag="zsp")
                    nc.vector.tensor_tensor(
                        zsp[:, 0:cc, :],
                        G[:, 0:cc, :],
                        asrc[:, None, :].to_broadcast([P, cc, OUT_DIM]),
                        mybir.AluOpType.mult,
                    )
                    zsc = ebuf.tile([P, CHUNK_COLS], F32, tag="zsc")
                    nc.vector.tensor_reduce(
                        zsc[:, 0:cc], zsp[:, 0:cc, :],
                        mybir.AxisListType.X, mybir.AluOpType.add,
                    )
                    score = ebuf.tile([P, CHUNK_COLS], F32, tag="score")
                    sc = score[:, 0:cc].rearrange("p (t w) -> p t w", w=w)
                    nc.vector.tensor_tensor(
                        sc,
                        zsc[:, 0:cc].rearrange("p (t w) -> p t w", w=w),
                        zdslots[h][:, t0:t0 + ntc, :].to_broadcast([P, ntc, w]),
                        mybir.AluOpType.add,
                    )
                    exf = ebuf.tile([P, CHUNK_COLS], F32, tag="exf")
                    nc.vector.scalar_tensor_tensor(
                        exf[:, 0:cc], score[:, 0:cc], 0.01, score[:, 0:cc],
                        op0=mybir.AluOpType.mult, op1=mybir.AluOpType.max,
                    )
                    nc.scalar.activation(
                        exf[:, 0:cc], exf[:, 0:cc],
                        mybir.ActivationFunctionType.Exp,
                    )
                    exm = ebuf.tile([P, CHUNK_COLS], BF16, tag="exm")
                    nc.vector.tensor_tensor(
                        exm[:, 0:cc], exf[:, 0:cc],
                        gmask_sb[h][:, c0:c0 + cc],
                        mybir.AluOpType.mult,
                    )
                    # denominators straight into agg col 128
                    nc.vector.tensor_reduce(
                        agg[h][:, t0:t0 + ntc, 128],
                        exm[:, 0:cc].rearrange("p (t w) -> p t w", w=w),
                        mybir.AxisListType.X,
                        mybir.AluOpType.add,
                    )
                    exz = ebuf.tile([P, CHUNK_COLS, OUT_DIM], BF16, tag="exz")
                    nc.vector.tensor_tensor(
                        exz[:, 0:cc, :],
                        G[:, 0:cc, 0:OUT_DIM],
                        exm[:, 0:cc, None].to_broadcast([P, cc, OUT_DIM]),
                        mybir.AluOpType.mult,
                    )
                    for ti in range(ntc):
                        ps = ps2.tile([P, OUT_DIM], F32, tag="aggps")
                        for r in range(w):
                            nc.tensor.matmul(
                                ps,
                                lhsT=ident,
                                rhs=exz[:, ti * w + r, :],
                                start=(r == 0),
                                stop=(r == w - 1),
                            )
                        nc.scalar.copy(agg[h][:, t0 + ti, 0:OUT_DIM], ps)

                nacc = [0]
                if USE_PREP:
                    # pipeline: first PREP_DEPTH preps, then rolling (prep, trig)
                    for k in range(min(PREP_DEPTH, nch)):
                        emit_prep(k)
                    # Tile does not thread the collective->table dep onto the
                    # deferred trigger.  Bounce it through an SP-engine probe
                    # read of `table` (which Tile does gate on the collective)
                    # and a Pool-engine copy of the probe tile, so the trigger
                    # (later in Pool program order) fires after the AllGather.
                    probe = metap.tile([P, 8], BF16, tag="tprobe")
                    nc.sync.dma_start(probe, table[0:P, 0:8])
                    pdummy = metap.tile([P, 8], BF16, tag="pdummy")
                    nc.gpsimd.tensor_copy(pdummy, probe)
                    nc.gpsimd.trigger_dma(count=None, queue_num=0)
                    for k in range(nch):
                        nk = k + PREP_DEPTH
                        if nk < nch:
                            emit_prep(nk)
                            nc.gpsimd.trigger_dma(count=None, queue_num=0)
                            if nk == acc0_prep_at:
                                # h0 compute is finishing around now: prep+fire
                                # its accumulator scatter on the scatter queue
                                emit_acc_scatter_prep(0)
                                nacc[0] += 1
                                nc.gpsimd.trigger_dma(count=None, queue_num=SCAT_Q)
                        emit_consumer(k)

                    emit_acc_scatter_prep(1)
                    nacc[0] += 1
                    nc.gpsimd.trigger_dma(count=None, queue_num=SCAT_Q)
                else:
                    for k in range(nch):
                        emit_prep(k)
                        emit_consumer(k)
                        if k == nch - 1 or allch[k + 1][0] != allch[k][0]:
                            emit_acc_scatter_prep(allch[k][0])

            # ---------------- phase 3: divide + output ----------------
            with tc.tile_pool(name="fin", bufs=1) as finp:
                if USE_PREP:
                    nc.sync.wait_ge(ssem, 16 * nacc[0])
                acc2 = finp.tile([P, NT, 2, SCAT_ELEMS], F32)
                av = acc.rearrange("(t p) d -> p t d", p=P)
                nc.sync.dma_start(acc2[:, :, 0, :], av[:, :, 0:SCAT_ELEMS])
                nc.sync.dma_start(
                    acc2[:, :, 1, :], av[:, :, ACC_OFF_H:ACC_OFF_H + SCAT_ELEMS]
                )
                accs = finp.tile([P, NT, SCAT_ELEMS], F32)
                nc.vector.tensor_tensor(
                    accs, acc2[:, :, 0, :], acc2[:, :, 1, :], mybir.AluOpType.add
                )
                den = finp.tile([P, NT], F32)
                nc.vector.tensor_copy(den, accs[:, :, 128])
                nc.vector.tensor_scalar(
                    den, den, 1e-30, None, mybir.AluOpType.max
                )
                rec = finp.tile([P, NT], F32)
                nc.vector.reciprocal(rec, den)
                res = finp.tile([P, NT, OUT_DIM], F32)
                nc.vector.tensor_tensor(
                    res,
                    accs[:, :, 0:OUT_DIM],
                    rec[:, :, None].to_broadcast([P, NT, OUT_DIM]),
                    mybir.AluOpType.mult,
                )
                nc.sync.dma_start(out.rearrange("(t p) d -> p t d", p=P), res)

    nc.finalize()
    return nc


def make_in_maps(cfg, struct, data, h, W_fc, a_attn):
    N, NPC, NPAD = cfg.N, cfg.NPC, cfg.NPAD
    h = np.asarray(h, np.float32)
    W_fc = np.asarray(W_fc, np.float32)
    a_attn = np.asarray(a_attn, np.float32)

    w_d = W_fc.T @ a_attn[OUT_DIM:]
    W_aug = np.concatenate(
        [W_fc.T, w_d[:, None]], axis=1
    ).astype(ml_dtypes.bfloat16)        # [256, 129]
    W_aug = np.ascontiguousarray(W_aug)

    ident = np.eye(P, dtype=ml_dtypes.bfloat16)
    asrc = np.tile(
        a_attn[:OUT_DIM].astype(ml_dtypes.bfloat16)[None, :], (P, 1)
    )

    hT_full = np.ascontiguousarray(h.T)  # [256, N]

    in_maps = []
    for c in range(NC):
        hT = np.zeros((IN_DIM, NPAD), ml_dtypes.bfloat16)
        hT[:, :NPC] = hT_full[:, c * NPC:(c + 1) * NPC].astype(ml_dtypes.bfloat16)
        gidx = np.stack([data[(c, 0)]["gidx"], data[(c, 1)]["gidx"]])
        gmask = np.stack([data[(c, 0)]["gmask"], data[(c, 1)]["gmask"]])
        mscat = np.stack([data[(c, 0)]["mscat"], data[(c, 1)]["mscat"]])
        zdscat = np.stack([data[(c, 0)]["zdscat"], data[(c, 1)]["zdscat"]])
        in_maps.append({
            "hT": np.ascontiguousarray(hT),
            "W_aug": W_aug,
            "ident": ident,
            "asrc": np.ascontiguousarray(asrc),
            "gidx": np.ascontiguousarray(gidx),
            "gmask": np.ascontiguousarray(gmask),
            "mscat": np.ascontiguousarray(mscat),
            "zdscat": np.ascontiguousarray(zdscat),
        })
    return in_maps


def run(h, src, dst, W_fc, a_attn, n_nodes=None, n_edges=None, trace=False):
    h = np.asarray(h, np.float32)
    cfg = Cfg(
        n_nodes if n_nodes is not None else h.shape[0],
        n_edges if n_edges is not None else np.asarray(src).shape[0],
    )
    struct, data = host_prep(cfg, src, dst)
    nc = build_program(cfg, struct)
    in_maps = make_in_maps(cfg, struct, data, h, W_fc, a_attn)
    results = run_bass_kernel_spmd(
        nc, in_maps, core_ids=list(range(NC)), trace=trace
    )
    outs = [r["out"] for r in results.results]
    full = np.concatenate([o[: cfg.NPC] for o in outs], axis=0).astype(np.float32)
    return full, results


def kernel(h, src, dst, W_fc, a_attn):
    full, _ = run(h, src, dst, W_fc, a_attn)
    return full


# revision 32
# speedup vs baseline: 1.1905x; 1.1905x over previous
"""GAT layer (gnn_message_passing) on 8 Trainium2 NeuronCores.

Strategy (dst-owner sharding, dst-major slot layout):
  - Node features z = h @ W.T are computed sharded in bf16: core c computes z
    for its 6250 "owned" destination nodes (zd = z@a_dst folded in as one
    extra matmul output column).
  - Each core builds 256B table rows (z in bf16[128]) and an AllGather
    replicates the full 50k-row table to every core.
  - Edges are bucketed by the owner of dst, so each core holds the COMPLETE
    in-edge set of its nodes: segment softmax is fully local.  Softmax uses
    the no-max-subtraction form (scores are O(10), exp stays in f32 range);
    alpha is never materialized: out = (sum_e ex_e * z[src_e]) / (sum_e ex_e).
  - Per core the owned nodes are sorted by in-degree and laid out dst-major:
    partition p = node slot, free column = edge rank.  dma_gather fetches
    z[src] rows straight into that layout.  zs[src] = z[src]@a_src is
    recomputed per edge from the gathered row by a DVE multiply+reduce
    (instead of shipping zs in the table); zd[dst] is a per-partition
    broadcast; the weighted segment-sum runs on the TensorEngine as a chain
    of identity-lhsT matmuls accumulating in PSUM.
  - GpSimd descriptor generation is the serial bottleneck (~7ns/index), so
    every SWDGE op runs as a prepare_only prep + trigger_dma: the zd-permute
    preps desc-gen during phase 1 (queue 1), gather desc-gen overlaps the
    AllGather and the DMA transfers, and only ONE accumulator scatter remains
    (h1): h0's aggregate is written with a plain strided DMA in slot order
    and the final output rows are un-permuted on the host.
  - Consumers gate on per-chunk descriptor-baked semaphores (8-deep
    rotation): Tile's own DMASW arm for prepped SWDGE fires at arm time,
    not DMA completion, so it cannot be used for data readiness.
  - dma_gather indices are int16, so the gather is split into two calls over
    the low/high halves of the table (separate slot layouts per half).
  - Zero-in-degree nodes get a fake self-edge on the host so out == z for
    them exactly (no select pass needed).
"""

import numpy as np
import ml_dtypes

import concourse.bass as bass
import concourse.mybir as mybir
import concourse.tile as tile
from concourse import bacc
from concourse import library_config
from concourse.bass import ts
from concourse.bass_utils import run_bass_kernel_spmd

F32 = mybir.dt.float32
BF16 = mybir.dt.bfloat16
I16 = mybir.dt.int16

NC = 8          # cores
P = 128         # partitions
IN_DIM = 256
OUT_DIM = 128
KCH = IN_DIM // P       # 2 k-chunks for the feature matmul
ROW_ELEMS = 128         # bf16 elems per table row (256 B)
ACC_STRIDE = 320        # f32 elems per accumulator row (1280 B)
ACC_OFF_H = 160         # h1 column offset within an acc row
SCAT_ELEMS = 129        # f32 elems per slot ([agg128 | den])
ZDP_STRIDE = 64         # f32 stride of the zd permute buffer (256 B)
CHUNK_COLS = 32         # max gather columns per chunk
PREP_DEPTH = 4          # gather preps issued before the first trigger
SCAT_Q = 0              # SWDGE queue for scatters (shared ring with gathers)


class Cfg:
    def __init__(self, n_nodes, n_edges):
        assert n_nodes % NC == 0
        self.N = n_nodes
        self.E = n_edges
        self.NPC = n_nodes // NC
        self.NPAD = ((self.NPC + P - 1) // P) * P
        self.NT = self.NPAD // P
        assert n_nodes % 2 == 0
        self.HALF = n_nodes // 2
        self.HALF_ROWS = (NC // 2) * self.NPAD  # table rows per half


def _wrap16(flat, dtype=np.int16):
    """flat[i] -> [128, len/16] with flat[i] at [i%16, i//16], replicated x8."""
    n = flat.shape[0]
    assert n % 16 == 0
    w = flat.reshape(n // 16, 16).T.astype(dtype)  # [16, n/16]
    return np.tile(w, (8, 1))


def host_prep(cfg, src, dst):
    """Build the common tile structure + per-core index/mask arrays."""
    N, NPC, NPAD, NT, HALF = cfg.N, cfg.NPC, cfg.NPAD, cfg.NT, cfg.HALF
    src = np.asarray(src, np.int64).copy()
    dst = np.asarray(dst, np.int64).copy()

    # fake self-edges for isolated (zero in-degree) nodes -> out == z exactly
    deg_tot = np.bincount(dst, minlength=N)
    iso = np.nonzero(deg_tot == 0)[0]
    if iso.size:
        src = np.concatenate([src, iso])
        dst = np.concatenate([dst, iso])

    owner = dst // NPC
    halves = (src >= HALF).astype(np.int64)

    # table row of a global node: owner block of NPAD rows + local index
    table_row = (src // NPC) * NPAD + (src % NPC)
    table_local = table_row - (halves * cfg.HALF_ROWS)
    assert table_local.max() < 32768 and table_local.min() >= 0

    per = {}
    deg_sorted_all = []
    for c in range(NC):
        for h in (0, 1):
            m = (owner == c) & (halves == h)
            es = table_local[m]               # gather index of each edge
            ed = dst[m] - c * NPC             # local dst node
            deg = np.bincount(ed, minlength=NPAD)
            order = np.argsort(-deg, kind="stable")  # node_of_slot [NPAD]
            sL = np.empty(NPAD, np.int64)
            sL[order] = np.arange(NPAD)              # slot of node
            per[(c, h)] = dict(es=es, ed=ed, deg=deg, order=order, sL=sL)
            deg_sorted_all.append(deg[order])

    # common tile widths
    W = np.zeros(NT, np.int64)
    for dsrt in deg_sorted_all:
        W = np.maximum(W, dsrt[::P][:NT])
    NTp = int(np.nonzero(W > 0)[0][-1]) + 1 if (W > 0).any() else 0
    W = W[:NTp]
    colstart = np.concatenate([[0], np.cumsum(W)]).astype(np.int64)
    C = int(colstart[-1])
    CP = C  # gather cols total per half

    # chunks: runs of equal W, capped at CHUNK_COLS columns
    chunks = []  # (t0, nt, W)
    t = 0
    while t < NTp:
        w = int(W[t])
        nt = 1
        while (
            t + nt < NTp
            and int(W[t + nt]) == w
            and (nt + 1) * w <= CHUNK_COLS
        ):
            nt += 1
        chunks.append((t, nt, w))
        t += nt

    # per-(core,half) flat arrays
    data = {}
    for c in range(NC):
        for h in (0, 1):
            d = per[(c, h)]
            es, ed, deg, order, sL = (
                d["es"], d["ed"], d["deg"], d["order"], d["sL"],
            )
            slot = sL[ed]
            tile_of = slot // P
            part_of = slot % P
            # rank of each edge within its dst node
            o = np.argsort(slot, kind="stable")
            slot_s = slot[o]
            es_s = es[o]
            tile_s = tile_of[o]
            part_s = part_of[o]
            counts = np.bincount(slot_s, minlength=NPAD)
            starts = np.concatenate([[0], np.cumsum(counts)])[:-1]
            rank = np.arange(slot_s.size) - starts[slot_s]
            keep = tile_s < NTp
            assert keep.all(), "edge landed outside processed tiles"
            cglob = colstart[tile_s] + rank
            assert (rank < W[tile_s]).all()
            pos = cglob * P + part_s

            flat_idx = np.zeros(CP * P, np.int16)
            flat_idx[pos] = es_s.astype(np.int16)
            mask = np.zeros((P, CP), np.float32)
            mask[part_s, cglob] = 1.0

            data[(c, h)] = dict(
                gidx=_wrap16(flat_idx),
                gmask=mask,
                zdscat=_wrap16(sL.astype(np.int16)),
            )
        # h1 agg scatter: slot1 s holds node order1[s]; its acc row (slot0
        # order) is sL0[order1[s]].
        sL0 = per[(c, 0)]["sL"]
        order1 = per[(c, 1)]["order"]
        data[(c, 1)]["mscat"] = _wrap16(sL0[order1][: NTp * P].astype(np.int16))
        # host-side output un-permute: node n -> acc/out row sL0[n]
        data[(c, 0)]["outperm"] = per[(c, 0)]["sL"][:NPC].copy()

    struct = dict(W=W, NTp=NTp, colstart=colstart, C=C, chunks=chunks)
    return struct, data


def build_program(cfg, struct):
    NPAD, NT, NTp, C = cfg.NPAD, cfg.NT, struct["NTp"], struct["C"]
    W, colstart, chunks = struct["W"], struct["colstart"], struct["chunks"]

    nc = bacc.Bacc(
        "TRN2", target_bir_lowering=False, debug=False, num_devices=NC,
        num_swdge_queues=1,
        dynamic_dma_scratch_size=32768,
    )

    # I/O
    hT = nc.dram_tensor("hT", [IN_DIM, NPAD], BF16, kind="ExternalInput").ap()
    W_aug = nc.dram_tensor("W_aug", [IN_DIM, 129], BF16, kind="ExternalInput").ap()
    ident_in = nc.dram_tensor("ident", [P, P], BF16, kind="ExternalInput").ap()
    asrc_in = nc.dram_tensor("asrc", [P, OUT_DIM], BF16, kind="ExternalInput").ap()
    gidx_in = nc.dram_tensor("gidx", [2, P, C * 8], I16, kind="ExternalInput").ap()
    gmask_in = nc.dram_tensor("gmask", [2, P, C], F32, kind="ExternalInput").ap()
    mscat_in = nc.dram_tensor("mscat", [P, NTp * 8], I16, kind="ExternalInput").ap()
    zdscat_in = nc.dram_tensor("zdscat", [2, P, NPAD // 16], I16, kind="ExternalInput").ap()

    out = nc.dram_tensor("out", [NPAD, OUT_DIM], F32, kind="ExternalOutput").ap()
    acc = nc.dram_tensor("acc", [NPAD, ACC_STRIDE], F32, kind="ExternalOutput").ap()
    zdperm = nc.dram_tensor("zdperm", [2 * NPAD, ZDP_STRIDE], F32, kind="ExternalOutput").ap()

    table_own = nc.dram_tensor("table_own", [NPAD, ROW_ELEMS], BF16, kind="Internal").ap()
    table = nc.dram_tensor(
        "table", [NC * NPAD, ROW_ELEMS], BF16, kind="Internal", addr_space="Shared"
    ).ap()

    nc.gpsimd.load_library(library_config.mlp)

    gsems = [nc.alloc_semaphore(f"gdma{i}") for i in range(8)]
    zsem = nc.alloc_semaphore("zd_dma")
    ssem = nc.alloc_semaphore("acc_dma")

    with tile.TileContext(nc) as tc:
        with tc.tile_pool(name="const", bufs=1) as constp:
            ident = constp.tile([P, P], BF16)
            nc.sync.dma_start(ident, ident_in)
            asrc = constp.tile([P, OUT_DIM], BF16)
            nc.sync.dma_start(asrc, asrc_in)

            # zd destination lives in the long-lived pool: its deferred read
            # (at zd trigger time) must not race later pool reuse.
            zd_sb = constp.tile([P, NT, 1], F32)

            # zd-permute preps: desc-gen runs during phase 1 on queue 1;
            # transfers fire (trigger below) once zd_sb is written.
            zdsc_sb = []
            for h in (0, 1):
                zdsc = constp.tile([P, NPAD // 16], I16, tag=f"zdsc{h}")
                nc.sync.dma_start(zdsc, zdscat_in[h])
                zdsc_sb.append(zdsc)
            # ---------------- phase 1: z / zd + table ----------------
            with (
                tc.tile_pool(name="ph1c", bufs=1) as ph1c,
                tc.tile_pool(name="ph1", bufs=6) as ph1,
                tc.tile_pool(name="ph1ps", bufs=4, space="PSUM") as ph1ps,
            ):
                wsb = ph1c.tile([P, KCH, 129], BF16)
                nc.sync.dma_start(wsb, W_aug.rearrange("(ko ki) m -> ki ko m", ki=P))

                hsb = ph1c.tile([P, KCH, NPAD], BF16)
                nc.sync.dma_start(hsb, hT.rearrange("(ko ki) n -> ki ko n", ki=P))

                for t in range(NT):
                    ps = ph1ps.tile([P, 129], F32)
                    for k in range(KCH):
                        nc.tensor.matmul(
                            ps,
                            lhsT=hsb[:, k, ts(t, P)],
                            rhs=wsb[:, k, :],
                            start=(k == 0),
                            stop=(k == KCH - 1),
                        )
                    row = ph1.tile([P, ROW_ELEMS], BF16, tag="rowbf")
                    nc.scalar.copy(row, ps[:, 0:OUT_DIM])
                    nc.vector.tensor_copy(zd_sb[:, t, :], ps[:, 128:129])
                    nc.sync.dma_start(table_own[ts(t, P), :], row)

                from concourse.instruction_name_ordered_set import (
                    InstructionNameOrderedSet,
                )
                with tc.high_priority():
                    coll = nc.gpsimd.collective_compute(
                        "AllGather",
                        mybir.AluOpType.bypass,
                        replica_groups=[list(range(NC))],
                        ins=[table_own],
                        outs=[table],
                    )
                # zd permute: plain scatters ordered AFTER the collective
                # trigger (nosync edge) so the mesh starts first; their
                # desc-gen+transfers then hide under the mesh window.
                for h in (0, 1):
                    scat = nc.gpsimd.dma_scatter_add(
                        out_ap=zdperm[h * NPAD:(h + 1) * NPAD, 0:1],
                        in_ap=zd_sb,
                        idxs_ap=zdsc_sb[h],
                        num_idxs=NPAD,
                        num_idxs_reg=NPAD,
                        elem_size=1,
                        elem_step=ZDP_STRIDE,
                        single_packet=NPAD <= 1024,
                    )
                    deps = InstructionNameOrderedSet()
                    deps.add(coll.ins.name)
                    scat.ins.add_nosync_dependencies_from(deps)

            # ---------------- phase 2: edges ----------------
            with (
                tc.tile_pool(name="meta", bufs=1) as metap,
                tc.tile_pool(name="gbuf", bufs=PREP_DEPTH + 2) as gbuf,
                tc.tile_pool(name="ebuf", bufs=3) as ebuf,
                tc.tile_pool(name="aggb", bufs=1) as aggb,
                tc.tile_pool(name="ps2", bufs=6, space="PSUM") as ps2,
            ):
                zdslots = []
                gidx_sb = []
                gmask_sb = []
                for h in (0, 1):
                    z = metap.tile([P, NT, 1], F32, tag=f"zds{h}")
                    nc.sync.dma_start(
                        z,
                        zdperm[h * NPAD:(h + 1) * NPAD, :]
                        .rearrange("(t p) d -> p t d", p=P)[:, :, 0:1],
                    )
                    zdslots.append(z)
                    g = metap.tile([P, C * 8], I16, tag=f"gidx{h}")
                    nc.sync.dma_start(g, gidx_in[h])
                    gidx_sb.append(g)
                    m = metap.tile([P, C], F32, tag=f"gmask{h}")
                    nc.sync.dma_start(m, gmask_in[h])
                    gmask_sb.append(m)
                mscat_sb = metap.tile([P, NTp * 8], I16)
                nc.sync.dma_start(mscat_sb, mscat_in)

                agg0 = aggb.tile([P, NTp, SCAT_ELEMS], F32, tag="agg0")
                agg1 = aggb.tile([P, NTp, SCAT_ELEMS], F32, tag="agg1")
                agg = {0: agg0, 1: agg1}

                # flat chunk list across both halves
                allch = [(h, t0, ntc, w) for h in (0, 1) for (t0, ntc, w) in chunks]
                nch = len(allch)
                gtiles = [None] * nch

                def emit_prep(k):
                    h, t0, ntc, w = allch[k]
                    cc = ntc * w
                    c0 = int(colstart[t0])
                    G = gbuf.tile([P, CHUNK_COLS, ROW_ELEMS], BF16, tag="G")
                    gp = nc.gpsimd.dma_gather(
                        out_ap=G[:, 0:cc, :],
                        in_ap=table[h * cfg.HALF_ROWS:(h + 1) * cfg.HALF_ROWS, :],
                        idxs_ap=gidx_sb[h][:, c0 * 8:(c0 + cc) * 8],
                        num_idxs=cc * P,
                        num_idxs_reg=cc * P,
                        elem_size=ROW_ELEMS,
                        single_packet=cc * P <= 1024,
                        prepare_only=True,
                        sem=gsems[k % 8],
                        queue_num=0,
                    )
                    gtiles[k] = G
                    return gp

                def emit_consumer(k):
                    h, t0, ntc, w = allch[k]
                    cc = ntc * w
                    c0 = int(colstart[t0])
                    G = gtiles[k]
                    # per-chunk descriptor-baked gate; rotation 8 exceeds the
                    # G-buffer WAR window and per-queue ring FIFO order makes
                    # the cumulative target exact.
                    nc.vector.wait_ge(gsems[k % 8], 16 * (k // 8 + 1))
                    # zs[src] per edge: dot(gathered z row, a_src)
                    zsp = ebuf.tile([P, CHUNK_COLS, OUT_DIM], BF16, tag="zsp")
                    nc.vector.tensor_tensor(
                        zsp[:, 0:cc, :],
                        G[:, 0:cc, :],
                        asrc[:, None, :].to_broadcast([P, cc, OUT_DIM]),
                        mybir.AluOpType.mult,
                    )
                    zsc = ebuf.tile([P, CHUNK_COLS], F32, tag="zsc")
                    nc.vector.tensor_reduce(
                        zsc[:, 0:cc], zsp[:, 0:cc, :],
                        mybir.AxisListType.X, mybir.AluOpType.add,
                    )
                    score = ebuf.tile([P, CHUNK_COLS], F32, tag="score")
                    sc = score[:, 0:cc].rearrange("p (t w) -> p t w", w=w)
                    nc.vector.tensor_tensor(
                        sc,
                        zsc[:, 0:cc].rearrange("p (t w) -> p t w", w=w),
                        zdslots[h][:, t0:t0 + ntc, :].to_broadcast([P, ntc, w]),
                        mybir.AluOpType.add,
                    )
                    exf = ebuf.tile([P, CHUNK_COLS], F32, tag="exf")
                    nc.vector.scalar_tensor_tensor(
                        exf[:, 0:cc], score[:, 0:cc], 0.01, score[:, 0:cc],
                        op0=mybir.AluOpType.mult, op1=mybir.AluOpType.max,
                    )
                    nc.scalar.activation(
                        exf[:, 0:cc], exf[:, 0:cc],
                        mybir.ActivationFunctionType.Exp,
                    )
                    exm = ebuf.tile([P, CHUNK_COLS], BF16, tag="exm")
                    nc.vector.tensor_tensor(
                        exm[:, 0:cc], exf[:, 0:cc],
                        gmask_sb[h][:, c0:c0 + cc],
                        mybir.AluOpType.mult,
                    )
                    # denominators straight into agg col 128
                    nc.vector.tensor_reduce(
                        agg[h][:, t0:t0 + ntc, 128],
                        exm[:, 0:cc].rearrange("p (t w) -> p t w", w=w),
                        mybir.AxisListType.X,
                        mybir.AluOpType.add,
                    )
                    exz = ebuf.tile([P, CHUNK_COLS, OUT_DIM], BF16, tag="exz")
                    nc.vector.tensor_tensor(
                        exz[:, 0:cc, :],
                        G[:, 0:cc, 0:OUT_DIM],
                        exm[:, 0:cc, None].to_broadcast([P, cc, OUT_DIM]),
                        mybir.AluOpType.mult,
                    )
                    for ti in range(ntc):
                        ps = ps2.tile([P, OUT_DIM], F32, tag="aggps")
                        for r in range(w):
                            nc.tensor.matmul(
                                ps,
                                lhsT=ident,
                                rhs=exz[:, ti * w + r, :],
                                start=(r == 0),
                                stop=(r == w - 1),
                            )
                        nc.scalar.copy(agg[h][:, t0 + ti, 0:OUT_DIM], ps)

                # pipeline: first PREP_DEPTH preps, then rolling (prep, trig)
                first_preps = InstructionNameOrderedSet()
                for k in range(min(PREP_DEPTH, nch)):
                    first_preps.add(emit_prep(k).ins.name)
                # Tile does not thread the collective->table dep onto the
                # deferred trigger.  Bounce it through an SP-engine probe
                # read of `table` (which Tile does gate on the collective)
                # and a Pool-engine copy of the probe tile.  Ordering edges:
                # gate after the first preps (their desc-gen runs during the
                # mesh), trigger after the gate.
                probe = metap.tile([P, 8], BF16, tag="tprobe")
                nc.sync.dma_start(probe, table[0:P, 0:8])
                pdummy = metap.tile([P, 8], BF16, tag="pdummy")
                gate = nc.gpsimd.tensor_copy(pdummy, probe)
                gate.ins.add_nosync_dependencies_from(first_preps)
                trig0 = nc.gpsimd.trigger_dma(count=None, queue_num=0)
                gdeps = InstructionNameOrderedSet()
                gdeps.add(gate.ins.name)
                trig0.ins.add_nosync_dependencies_from(gdeps)
                for k in range(nch):
                    nk = k + PREP_DEPTH
                    if nk < nch:
                        emit_prep(nk)
                        nc.gpsimd.trigger_dma(count=None, queue_num=0)
                    emit_consumer(k)
                    if k == len(chunks) - 1:
                        # h0 done: plain strided write of agg0 in slot order
                        nc.sync.dma_start(
                            acc.rearrange("(t p) d -> p t d", p=P)[:, :NTp, 0:SCAT_ELEMS],
                            agg0,
                        )

                # h1 accumulator scatter (slot1 -> slot0 rows)
                nc.gpsimd.dma_scatter_add(
                    out_ap=acc[:, ACC_OFF_H:ACC_OFF_H + SCAT_ELEMS],
                    in_ap=agg1,
                    idxs_ap=mscat_sb,
                    num_idxs=NTp * P,
                    num_idxs_reg=NTp * P,
                    elem_size=SCAT_ELEMS,
                    elem_step=ACC_STRIDE,
                    single_packet=NTp * P <= 1024,
                    prepare_only=True,
                    sem=ssem,
                    queue_num=SCAT_Q,
                )
                nc.gpsimd.trigger_dma(count=None, queue_num=SCAT_Q)

            # ---------------- phase 3: divide + output ----------------
            with tc.tile_pool(name="fin", bufs=1) as finp:
                nc.sync.wait_ge(ssem, 16)
                acc2 = finp.tile([P, NT, 2, SCAT_ELEMS], F32)
                av = acc.rearrange("(t p) d -> p t d", p=P)
                nc.sync.dma_start(acc2[:, :, 0, :], av[:, :, 0:SCAT_ELEMS])
                nc.sync.dma_start(
                    acc2[:, :, 1, :], av[:, :, ACC_OFF_H:ACC_OFF_H + SCAT_ELEMS]
                )
                accs = finp.tile([P, NT, SCAT_ELEMS], F32)
                nc.vector.tensor_tensor(
                    accs, acc2[:, :, 0, :], acc2[:, :, 1, :], mybir.AluOpType.add
                )
                den = finp.tile([P, NT], F32)
                nc.vector.tensor_copy(den, accs[:, :, 128])
                nc.vector.tensor_scalar(
                    den, den, 1e-30, None, mybir.AluOpType.max
                )
                rec = finp.tile([P, NT], F32)
                nc.vector.reciprocal(rec, den)
                res = finp.tile([P, NT, OUT_DIM], F32)
                nc.vector.tensor_tensor(
                    res,
                    accs[:, :, 0:OUT_DIM],
                    rec[:, :, None].to_broadcast([P, NT, OUT_DIM]),
                    mybir.AluOpType.mult,
                )
                nc.sync.dma_start(out.rearrange("(t p) d -> p t d", p=P), res)

    nc.finalize()
    return nc


def make_in_maps(cfg, struct, data, h, W_fc, a_attn):
    N, NPC, NPAD = cfg.N, cfg.NPC, cfg.NPAD
    h = np.asarray(h, np.float32)
    W_fc = np.asarray(W_fc, np.float32)
    a_attn = np.asarray(a_attn, np.float32)

    w_d = W_fc.T @ a_attn[OUT_DIM:]
    W_aug = np.concatenate(
        [W_fc.T, w_d[:, None]], axis=1
    ).astype(ml_dtypes.bfloat16)        # [256, 129]
    W_aug = np.ascontiguousarray(W_aug)

    ident = np.eye(P, dtype=ml_dtypes.bfloat16)
    asrc = np.tile(
        a_attn[:OUT_DIM].astype(ml_dtypes.bfloat16)[None, :], (P, 1)
    )

    hT_full = np.ascontiguousarray(h.T)  # [256, N]

    in_maps = []
    for c in range(NC):
        hT = np.zeros((IN_DIM, NPAD), ml_dtypes.bfloat16)
        hT[:, :NPC] = hT_full[:, c * NPC:(c + 1) * NPC].astype(ml_dtypes.bfloat16)
        gidx = np.stack([data[(c, 0)]["gidx"], data[(c, 1)]["gidx"]])
        gmask = np.stack([data[(c, 0)]["gmask"], data[(c, 1)]["gmask"]])
        zdscat = np.stack([data[(c, 0)]["zdscat"], data[(c, 1)]["zdscat"]])
        in_maps.append({
            "hT": np.ascontiguousarray(hT),
            "W_aug": W_aug,
            "ident": ident,
            "asrc": np.ascontiguousarray(asrc),
            "gidx": np.ascontiguousarray(gidx),
            "gmask": np.ascontiguousarray(gmask),
            "mscat": np.ascontiguousarray(data[(c, 1)]["mscat"]),
            "zdscat": np.ascontiguousarray(zdscat),
        })
    return in_maps


def run(h, src, dst, W_fc, a_attn, n_nodes=None, n_edges=None, trace=False):
    h = np.asarray(h, np.float32)
    cfg = Cfg(
        n_nodes if n_nodes is not None else h.shape[0],
        n_edges if n_edges is not None else np.asarray(src).shape[0],
    )
    struct, data = host_prep(cfg, src, dst)
    nc = build_program(cfg, struct)
    in_maps = make_in_maps(cfg, struct, data, h, W_fc, a_attn)
    results = run_bass_kernel_spmd(
        nc, in_maps, core_ids=list(range(NC)), trace=trace
    )
    outs = []
    for c in range(NC):
        o = results.results[c]["out"]
        outs.append(o[data[(c, 0)]["outperm"]])  # slot0 order -> node order
    full = np.concatenate(outs, axis=0).astype(np.float32)
    return full, results


def kernel(h, src, dst, W_fc, a_attn):
    full, _ = run(h, src, dst, W_fc, a_attn)
    return full


# revision 33
# speedup vs baseline: 1.3009x; 1.0927x over previous
"""GAT layer (gnn_message_passing) on 8 Trainium2 NeuronCores.

Strategy (dst-owner sharding, dst-major slot layout):
  - Node features z = h @ W.T are computed sharded in bf16: core c computes z
    for its 6250 "owned" destination nodes (zd = z@a_dst folded in as one
    extra matmul output column).
  - Each core builds 256B table rows (z in bf16[128]) and an AllGather
    replicates the full 50k-row table to every core.
  - Edges are bucketed by the owner of dst, so each core holds the COMPLETE
    in-edge set of its nodes: segment softmax is fully local.  Softmax uses
    the no-max-subtraction form (scores are O(10), exp stays in f32 range);
    alpha is never materialized: out = (sum_e ex_e * z[src_e]) / (sum_e ex_e).
  - Per core the owned nodes are sorted by in-degree and laid out dst-major:
    partition p = node slot, free column = edge rank.  dma_gather fetches
    z[src] rows straight into that layout.  zs[src] = z[src]@a_src is
    recomputed per edge from the gathered row by a DVE multiply+reduce
    (instead of shipping zs in the table); zd[dst] is a per-partition
    broadcast; the weighted segment-sum runs on the TensorEngine as a chain
    of identity-lhsT matmuls accumulating in PSUM.
  - GpSimd descriptor generation is the serial bottleneck (~7ns/index), so
    every SWDGE op runs as a prepare_only prep + trigger_dma: the zd-permute
    preps desc-gen during phase 1 (queue 1), gather desc-gen overlaps the
    AllGather and the DMA transfers, and only ONE accumulator scatter remains
    (h1): h0's aggregate is written with a plain strided DMA in slot order
    and the final output rows are un-permuted on the host.
  - Consumers gate on per-chunk descriptor-baked semaphores (8-deep
    rotation): Tile's own DMASW arm for prepped SWDGE fires at arm time,
    not DMA completion, so it cannot be used for data readiness.
  - dma_gather indices are int16, so the gather is split into two calls over
    the low/high halves of the table (separate slot layouts per half).
  - Zero-in-degree nodes get a fake self-edge on the host so out == z for
    them exactly (no select pass needed).
"""

import numpy as np
import ml_dtypes

import concourse.bass as bass
import concourse.mybir as mybir
import concourse.tile as tile
from concourse import bacc
from concourse import library_config
from concourse.bass import ts
from concourse.bass_utils import run_bass_kernel_spmd

F32 = mybir.dt.float32
BF16 = mybir.dt.bfloat16
I16 = mybir.dt.int16

NC = 8          # cores
P = 128         # partitions
IN_DIM = 256
OUT_DIM = 128
KCH = IN_DIM // P       # 2 k-chunks for the feature matmul
ROW_ELEMS = 128         # bf16 elems per table row (256 B)
ACC_STRIDE = 320        # f32 elems per accumulator row (1280 B)
ACC_OFF_H = 160         # h1 column offset within an acc row
SCAT_ELEMS = 129        # f32 elems per slot ([agg128 | den])
ZDP_STRIDE = 64         # f32 stride of the zd permute buffer (256 B)
CHUNK_COLS = 32         # max gather columns per chunk
PREP_DEPTH = 4          # gather preps issued before the first trigger
SCAT_Q = 0              # SWDGE queue for scatters (shared ring with gathers)


class Cfg:
    def __init__(self, n_nodes, n_edges):
        assert n_nodes % NC == 0
        self.N = n_nodes
        self.E = n_edges
        self.NPC = n_nodes // NC
        self.NPAD = ((self.NPC + P - 1) // P) * P
        self.NT = self.NPAD // P
        assert n_nodes % 2 == 0
        self.HALF = n_nodes // 2
        self.HALF_ROWS = (NC // 2) * self.NPAD  # table rows per half


def _wrap16(flat, dtype=np.int16):
    """flat[i] -> [128, len/16] with flat[i] at [i%16, i//16], replicated x8."""
    n = flat.shape[0]
    assert n % 16 == 0
    w = flat.reshape(n // 16, 16).T.astype(dtype)  # [16, n/16]
    return np.tile(w, (8, 1))


def host_prep(cfg, src, dst):
    """Build the common tile structure + per-core index/mask arrays."""
    N, NPC, NPAD, NT, HALF = cfg.N, cfg.NPC, cfg.NPAD, cfg.NT, cfg.HALF
    src = np.asarray(src, np.int64).copy()
    dst = np.asarray(dst, np.int64).copy()

    # fake self-edges for isolated (zero in-degree) nodes -> out == z exactly
    deg_tot = np.bincount(dst, minlength=N)
    iso = np.nonzero(deg_tot == 0)[0]
    if iso.size:
        src = np.concatenate([src, iso])
        dst = np.concatenate([dst, iso])

    owner = dst // NPC
    halves = (src >= HALF).astype(np.int64)

    # table row of a global node: owner block of NPAD rows + local index
    table_row = (src // NPC) * NPAD + (src % NPC)
    table_local = table_row - (halves * cfg.HALF_ROWS)
    assert table_local.max() < 32768 and table_local.min() >= 0

    per = {}
    deg_sorted_all = []
    for c in range(NC):
        for h in (0, 1):
            m = (owner == c) & (halves == h)
            es = table_local[m]               # gather index of each edge
            ed = dst[m] - c * NPC             # local dst node
            deg = np.bincount(ed, minlength=NPAD)
            order = np.argsort(-deg, kind="stable")  # node_of_slot [NPAD]
            sL = np.empty(NPAD, np.int64)
            sL[order] = np.arange(NPAD)              # slot of node
            per[(c, h)] = dict(es=es, ed=ed, deg=deg, order=order, sL=sL)
            deg_sorted_all.append(deg[order])

    # common tile widths
    W = np.zeros(NT, np.int64)
    for dsrt in deg_sorted_all:
        W = np.maximum(W, dsrt[::P][:NT])
    NTp = int(np.nonzero(W > 0)[0][-1]) + 1 if (W > 0).any() else 0
    W = W[:NTp]
    colstart = np.concatenate([[0], np.cumsum(W)]).astype(np.int64)
    C = int(colstart[-1])
    CP = C  # gather cols total per half

    # chunks: runs of equal W, capped at CHUNK_COLS columns
    chunks = []  # (t0, nt, W)
    t = 0
    while t < NTp:
        w = int(W[t])
        nt = 1
        while (
            t + nt < NTp
            and int(W[t + nt]) == w
            and (nt + 1) * w <= CHUNK_COLS
        ):
            nt += 1
        chunks.append((t, nt, w))
        t += nt

    # per-(core,half) flat arrays
    data = {}
    for c in range(NC):
        for h in (0, 1):
            d = per[(c, h)]
            es, ed, deg, order, sL = (
                d["es"], d["ed"], d["deg"], d["order"], d["sL"],
            )
            slot = sL[ed]
            tile_of = slot // P
            part_of = slot % P
            # rank of each edge within its dst node
            o = np.argsort(slot, kind="stable")
            slot_s = slot[o]
            es_s = es[o]
            tile_s = tile_of[o]
            part_s = part_of[o]
            counts = np.bincount(slot_s, minlength=NPAD)
            starts = np.concatenate([[0], np.cumsum(counts)])[:-1]
            rank = np.arange(slot_s.size) - starts[slot_s]
            keep = tile_s < NTp
            assert keep.all(), "edge landed outside processed tiles"
            cglob = colstart[tile_s] + rank
            assert (rank < W[tile_s]).all()
            pos = cglob * P + part_s

            flat_idx = np.zeros(CP * P, np.int16)
            flat_idx[pos] = es_s.astype(np.int16)
            mask = np.zeros((P, CP), np.float32)
            mask[part_s, cglob] = 1.0

            data[(c, h)] = dict(
                gidx=_wrap16(flat_idx),
                gmask=mask,
                zdscat=_wrap16(sL.astype(np.int16)),
            )
        # h1 agg scatter: slot1 s holds node order1[s]; its acc row (slot0
        # order) is sL0[order1[s]].
        sL0 = per[(c, 0)]["sL"]
        order1 = per[(c, 1)]["order"]
        data[(c, 1)]["mscat"] = _wrap16(sL0[order1][: NTp * P].astype(np.int16))
        # host-side output un-permute: node n -> acc/out row sL0[n]
        data[(c, 0)]["outperm"] = per[(c, 0)]["sL"][:NPC].copy()

    struct = dict(W=W, NTp=NTp, colstart=colstart, C=C, chunks=chunks)
    return struct, data


def build_program(cfg, struct):
    NPAD, NT, NTp, C = cfg.NPAD, cfg.NT, struct["NTp"], struct["C"]
    W, colstart, chunks = struct["W"], struct["colstart"], struct["chunks"]

    nc = bacc.Bacc(
        "TRN2", target_bir_lowering=False, debug=False, num_devices=NC,
        num_swdge_queues=1,
        dynamic_dma_scratch_size=32768,
    )

    # I/O
    hT = nc.dram_tensor("hT", [IN_DIM, NPAD], BF16, kind="ExternalInput").ap()
    W_aug = nc.dram_tensor("W_aug", [IN_DIM, 129], BF16, kind="ExternalInput").ap()
    ident_in = nc.dram_tensor("ident", [P, P], BF16, kind="ExternalInput").ap()
    asrc_in = nc.dram_tensor("asrc", [P, OUT_DIM], BF16, kind="ExternalInput").ap()
    gidx_in = nc.dram_tensor("gidx", [2, P, C * 8], I16, kind="ExternalInput").ap()
    gmask_in = nc.dram_tensor("gmask", [2, P, C], F32, kind="ExternalInput").ap()
    mscat_in = nc.dram_tensor("mscat", [P, NTp * 8], I16, kind="ExternalInput").ap()
    zdscat_in = nc.dram_tensor("zdscat", [2, P, NPAD // 16], I16, kind="ExternalInput").ap()

    out = nc.dram_tensor("out", [NPAD, OUT_DIM], F32, kind="ExternalOutput").ap()
    acc = nc.dram_tensor("acc", [NPAD, ACC_STRIDE], F32, kind="ExternalOutput").ap()
    zdperm = nc.dram_tensor("zdperm", [2 * NPAD, ZDP_STRIDE], F32, kind="ExternalOutput").ap()

    table_own = nc.dram_tensor("table_own", [NPAD, ROW_ELEMS], BF16, kind="Internal").ap()
    table = nc.dram_tensor(
        "table", [NC * NPAD, ROW_ELEMS], BF16, kind="Internal", addr_space="Shared"
    ).ap()

    nc.gpsimd.load_library(library_config.mlp)

    gsems = [nc.alloc_semaphore(f"gdma{i}") for i in range(8)]
    zsem = nc.alloc_semaphore("zd_dma")
    ssem = nc.alloc_semaphore("acc_dma")

    with tile.TileContext(nc) as tc:
        with tc.tile_pool(name="const", bufs=1) as constp:
            ident = constp.tile([P, P], BF16)
            nc.sync.dma_start(ident, ident_in)
            asrc = constp.tile([P, OUT_DIM], BF16)
            nc.sync.dma_start(asrc, asrc_in)

            # zd destination lives in the long-lived pool: its deferred read
            # (at zd trigger time) must not race later pool reuse.
            zd_sb = constp.tile([P, NT, 1], F32)

            # zd-permute preps: desc-gen runs during phase 1 on queue 1;
            # transfers fire (trigger below) once zd_sb is written.
            zdsc_sb = []
            for h in (0, 1):
                zdsc = constp.tile([P, NPAD // 16], I16, tag=f"zdsc{h}")
                nc.sync.dma_start(zdsc, zdscat_in[h])
                zdsc_sb.append(zdsc)
            # ---------------- phase 1: z / zd + table ----------------
            with (
                tc.tile_pool(name="ph1c", bufs=1) as ph1c,
                tc.tile_pool(name="ph1", bufs=6) as ph1,
                tc.tile_pool(name="ph1ps", bufs=4, space="PSUM") as ph1ps,
            ):
                wsb = ph1c.tile([P, KCH, 129], BF16)
                nc.sync.dma_start(wsb, W_aug.rearrange("(ko ki) m -> ki ko m", ki=P))

                hsb = ph1c.tile([P, KCH, NPAD], BF16)
                nc.sync.dma_start(hsb, hT.rearrange("(ko ki) n -> ki ko n", ki=P))

                for t in range(NT):
                    ps = ph1ps.tile([P, 129], F32)
                    for k in range(KCH):
                        nc.tensor.matmul(
                            ps,
                            lhsT=hsb[:, k, ts(t, P)],
                            rhs=wsb[:, k, :],
                            start=(k == 0),
                            stop=(k == KCH - 1),
                        )
                    row = ph1.tile([P, ROW_ELEMS], BF16, tag="rowbf")
                    nc.scalar.copy(row, ps[:, 0:OUT_DIM])
                    nc.vector.tensor_copy(zd_sb[:, t, :], ps[:, 128:129])
                    nc.sync.dma_start(table_own[ts(t, P), :], row)

                from concourse.instruction_name_ordered_set import (
                    InstructionNameOrderedSet,
                )
                with tc.high_priority():
                    coll = nc.gpsimd.collective_compute(
                        "AllGather",
                        mybir.AluOpType.bypass,
                        replica_groups=[list(range(NC))],
                        ins=[table_own],
                        outs=[table],
                    )
                # zd permute: plain scatters ordered AFTER the collective
                # trigger (nosync edge) so the mesh starts first; their
                # desc-gen+transfers then hide under the mesh window.
                for h in (0, 1):
                    scat = nc.gpsimd.dma_scatter_add(
                        out_ap=zdperm[h * NPAD:(h + 1) * NPAD, 0:1],
                        in_ap=zd_sb,
                        idxs_ap=zdsc_sb[h],
                        num_idxs=NPAD,
                        num_idxs_reg=NPAD,
                        elem_size=1,
                        elem_step=ZDP_STRIDE,
                        single_packet=NPAD <= 1024,
                    )
                    deps = InstructionNameOrderedSet()
                    deps.add(coll.ins.name)
                    scat.ins.add_nosync_dependencies_from(deps)

            # ---------------- phase 2: edges ----------------
            with (
                tc.tile_pool(name="meta", bufs=1) as metap,
                tc.tile_pool(name="gbuf", bufs=PREP_DEPTH + 2) as gbuf,
                tc.tile_pool(name="ebuf", bufs=3) as ebuf,
                tc.tile_pool(name="aggb", bufs=1) as aggb,
                tc.tile_pool(name="ps2", bufs=6, space="PSUM") as ps2,
            ):
                zdslots = []
                gidx_sb = []
                gmask_sb = []
                for h in (0, 1):
                    z = metap.tile([P, NT, 1], F32, tag=f"zds{h}")
                    nc.sync.dma_start(
                        z,
                        zdperm[h * NPAD:(h + 1) * NPAD, :]
                        .rearrange("(t p) d -> p t d", p=P)[:, :, 0:1],
                    )
                    zdslots.append(z)
                    g = metap.tile([P, C * 8], I16, tag=f"gidx{h}")
                    nc.sync.dma_start(g, gidx_in[h])
                    gidx_sb.append(g)
                    m = metap.tile([P, C], F32, tag=f"gmask{h}")
                    nc.sync.dma_start(m, gmask_in[h])
                    gmask_sb.append(m)
                mscat_sb = metap.tile([P, NTp * 8], I16)
                nc.sync.dma_start(mscat_sb, mscat_in)

                agg0 = aggb.tile([P, NTp, SCAT_ELEMS], F32, tag="agg0")
                agg1 = aggb.tile([P, NTp, SCAT_ELEMS], F32, tag="agg1")
                agg = {0: agg0, 1: agg1}

                # flat chunk list across both halves
                allch = [(h, t0, ntc, w) for h in (0, 1) for (t0, ntc, w) in chunks]
                nch = len(allch)
                gtiles = [None] * nch

                def emit_prep(k):
                    h, t0, ntc, w = allch[k]
                    cc = ntc * w
                    c0 = int(colstart[t0])
                    G = gbuf.tile([P, CHUNK_COLS, ROW_ELEMS], BF16, tag="G")
                    gp = nc.gpsimd.dma_gather(
                        out_ap=G[:, 0:cc, :],
                        in_ap=table[h * cfg.HALF_ROWS:(h + 1) * cfg.HALF_ROWS, :],
                        idxs_ap=gidx_sb[h][:, c0 * 8:(c0 + cc) * 8],
                        num_idxs=cc * P,
                        num_idxs_reg=cc * P,
                        elem_size=ROW_ELEMS,
                        single_packet=cc * P <= 1024,
                        prepare_only=True,
                        sem=gsems[k % 8],
                        queue_num=0,
                    )
                    gtiles[k] = G
                    return gp

                def emit_consumer(k):
                    h, t0, ntc, w = allch[k]
                    cc = ntc * w
                    c0 = int(colstart[t0])
                    G = gtiles[k]
                    # per-chunk descriptor-baked gate; rotation 8 exceeds the
                    # G-buffer WAR window and per-queue ring FIFO order makes
                    # the cumulative target exact.
                    nc.vector.wait_ge(gsems[k % 8], 16 * (k // 8 + 1))
                    # zs[src] per edge: dot(gathered z row, a_src)
                    zsp = ebuf.tile([P, CHUNK_COLS, OUT_DIM], BF16, tag="zsp")
                    nc.vector.tensor_tensor(
                        zsp[:, 0:cc, :],
                        G[:, 0:cc, :],
                        asrc[:, None, :].to_broadcast([P, cc, OUT_DIM]),
                        mybir.AluOpType.mult,
                    )
                    zsc = ebuf.tile([P, CHUNK_COLS], F32, tag="zsc")
                    nc.vector.tensor_reduce(
                        zsc[:, 0:cc], zsp[:, 0:cc, :],
                        mybir.AxisListType.X, mybir.AluOpType.add,
                    )
                    score = ebuf.tile([P, CHUNK_COLS], F32, tag="score")
                    sc = score[:, 0:cc].rearrange("p (t w) -> p t w", w=w)
                    nc.vector.tensor_tensor(
                        sc,
                        zsc[:, 0:cc].rearrange("p (t w) -> p t w", w=w),
                        zdslots[h][:, t0:t0 + ntc, :].to_broadcast([P, ntc, w]),
                        mybir.AluOpType.add,
                    )
                    exf = ebuf.tile([P, CHUNK_COLS], F32, tag="exf")
                    nc.vector.scalar_tensor_tensor(
                        exf[:, 0:cc], score[:, 0:cc], 0.01, score[:, 0:cc],
                        op0=mybir.AluOpType.mult, op1=mybir.AluOpType.max,
                    )
                    nc.scalar.activation(
                        exf[:, 0:cc], exf[:, 0:cc],
                        mybir.ActivationFunctionType.Exp,
                    )
                    exm = ebuf.tile([P, CHUNK_COLS], BF16, tag="exm")
                    nc.vector.tensor_tensor(
                        exm[:, 0:cc], exf[:, 0:cc],
                        gmask_sb[h][:, c0:c0 + cc],
                        mybir.AluOpType.mult,
                    )
                    # denominators straight into agg col 128
                    nc.vector.tensor_reduce(
                        agg[h][:, t0:t0 + ntc, 128],
                        exm[:, 0:cc].rearrange("p (t w) -> p t w", w=w),
                        mybir.AxisListType.X,
                        mybir.AluOpType.add,
                    )
                    exz = ebuf.tile([P, CHUNK_COLS, OUT_DIM], BF16, tag="exz")
                    nc.vector.tensor_tensor(
                        exz[:, 0:cc, :],
                        G[:, 0:cc, 0:OUT_DIM],
                        exm[:, 0:cc, None].to_broadcast([P, cc, OUT_DIM]),
                        mybir.AluOpType.mult,
                    )
                    for ti in range(ntc):
                        ps = ps2.tile([P, OUT_DIM], F32, tag="aggps")
                        for r in range(w):
                            nc.tensor.matmul(
                                ps,
                                lhsT=ident,
                                rhs=exz[:, ti * w + r, :],
                                start=(r == 0),
                                stop=(r == w - 1),
                            )
                        nc.scalar.copy(agg[h][:, t0 + ti, 0:OUT_DIM], ps)

                # pipeline: first PREP_DEPTH preps, then rolling (prep, trig)
                first_preps = InstructionNameOrderedSet()
                for k in range(min(PREP_DEPTH, nch)):
                    first_preps.add(emit_prep(k).ins.name)
                # Tile does not thread the collective->table dep onto the
                # deferred trigger.  Bounce it through an SP-engine probe
                # read of `table` (which Tile does gate on the collective)
                # and a Pool-engine copy of the probe tile.  Ordering edges:
                # gate after the first preps (their desc-gen runs during the
                # mesh), trigger after the gate.
                probe = metap.tile([P, 8], BF16, tag="tprobe")
                nc.sync.dma_start(probe, table[0:P, 0:8])
                pdummy = metap.tile([P, 8], BF16, tag="pdummy")
                gate = nc.gpsimd.tensor_copy(pdummy, probe)
                gate.ins.add_nosync_dependencies_from(first_preps)
                trig0 = nc.gpsimd.trigger_dma(count=None, queue_num=0)
                gdeps = InstructionNameOrderedSet()
                gdeps.add(gate.ins.name)
                trig0.ins.add_nosync_dependencies_from(gdeps)
                for k in range(nch):
                    nk = k + PREP_DEPTH
                    if nk < nch:
                        emit_prep(nk)
                        # batch rolling triggers in pairs to halve per-op
                        # sequencer/sem overhead on the Pool queue
                        if (nk - PREP_DEPTH) % 2 == 1 or nk == nch - 1:
                            nc.gpsimd.trigger_dma(count=None, queue_num=0)
                    emit_consumer(k)
                    if k == len(chunks) - 1:
                        # h0 done: plain strided write of agg0 in slot order
                        nc.sync.dma_start(
                            acc.rearrange("(t p) d -> p t d", p=P)[:, :NTp, 0:SCAT_ELEMS],
                            agg0,
                        )

                # h1 accumulator scatter (slot1 -> slot0 rows)
                nc.gpsimd.dma_scatter_add(
                    out_ap=acc[:, ACC_OFF_H:ACC_OFF_H + SCAT_ELEMS],
                    in_ap=agg1,
                    idxs_ap=mscat_sb,
                    num_idxs=NTp * P,
                    num_idxs_reg=NTp * P,
                    elem_size=SCAT_ELEMS,
                    elem_step=ACC_STRIDE,
                    single_packet=NTp * P <= 1024,
                    prepare_only=True,
                    sem=ssem,
                    queue_num=SCAT_Q,
                )
                nc.gpsimd.trigger_dma(count=None, queue_num=SCAT_Q)

            # ---------------- phase 3: divide + output ----------------
            with tc.tile_pool(name="fin", bufs=1) as finp:
                nc.sync.wait_ge(ssem, 16)
                acc2 = finp.tile([P, NT, 2, SCAT_ELEMS], F32)
                av = acc.rearrange("(t p) d -> p t d", p=P)
                nc.sync.dma_start(acc2[:, :, 0, :], av[:, :, 0:SCAT_ELEMS])
                nc.sync.dma_start(
                    acc2[:, :, 1, :], av[:, :, ACC_OFF_H:ACC_OFF_H + SCAT_ELEMS]
                )
                accs = finp.tile([P, NT, SCAT_ELEMS], F32)
                nc.vector.tensor_tensor(
                    accs, acc2[:, :, 0, :], acc2[:, :, 1, :], mybir.AluOpType.add
                )
                den = finp.tile([P, NT], F32)
                nc.vector.tensor_copy(den, accs[:, :, 128])
                nc.vector.tensor_scalar(
                    den, den, 1e-30, None, mybir.AluOpType.max
                )
                rec = finp.tile([P, NT], F32)
                nc.vector.reciprocal(rec, den)
                res = finp.tile([P, NT, OUT_DIM], F32)
                nc.vector.tensor_tensor(
                    res,
                    accs[:, :, 0:OUT_DIM],
                    rec[:, :, None].to_broadcast([P, NT, OUT_DIM]),
                    mybir.AluOpType.mult,
                )
                nc.sync.dma_start(out.rearrange("(t p) d -> p t d", p=P), res)

    nc.finalize()
    return nc


def make_in_maps(cfg, struct, data, h, W_fc, a_attn):
    N, NPC, NPAD = cfg.N, cfg.NPC, cfg.NPAD
    h = np.asarray(h, np.float32)
    W_fc = np.asarray(W_fc, np.float32)
    a_attn = np.asarray(a_attn, np.float32)

    w_d = W_fc.T @ a_attn[OUT_DIM:]
    W_aug = np.concatenate(
        [W_fc.T, w_d[:, None]], axis=1
    ).astype(ml_dtypes.bfloat16)        # [256, 129]
    W_aug = np.ascontiguousarray(W_aug)

    ident = np.eye(P, dtype=ml_dtypes.bfloat16)
    asrc = np.tile(
        a_attn[:OUT_DIM].astype(ml_dtypes.bfloat16)[None, :], (P, 1)
    )

    hT_full = np.ascontiguousarray(h.T)  # [256, N]

    in_maps = []
    for c in range(NC):
        hT = np.zeros((IN_DIM, NPAD), ml_dtypes.bfloat16)
        hT[:, :NPC] = hT_full[:, c * NPC:(c + 1) * NPC].astype(ml_dtypes.bfloat16)
        gidx = np.stack([data[(c, 0)]["gidx"], data[(c, 1)]["gidx"]])
        gmask = np.stack([data[(c, 0)]["gmask"], data[(c, 1)]["gmask"]])
        zdscat = np.stack([data[(c, 0)]["zdscat"], data[(c, 1)]["zdscat"]])
        in_maps.append({
            "hT": np.ascontiguousarray(hT),
            "W_aug": W_aug,
            "ident": ident,
            "asrc": np.ascontiguousarray(asrc),
            "gidx": np.ascontiguousarray(gidx),
            "gmask": np.ascontiguousarray(gmask),
            "mscat": np.ascontiguousarray(data[(c, 1)]["mscat"]),
            "zdscat": np.ascontiguousarray(zdscat),
        })
    return in_maps


def run(h, src, dst, W_fc, a_attn, n_nodes=None, n_edges=None, trace=False):
    h = np.asarray(h, np.float32)
    cfg = Cfg(
        n_nodes if n_nodes is not None else h.shape[0],
        n_edges if n_edges is not None else np.asarray(src).shape[0],
    )
    struct, data = host_prep(cfg, src, dst)
    nc = build_program(cfg, struct)
    in_maps = make_in_maps(cfg, struct, data, h, W_fc, a_attn)
    results = run_bass_kernel_spmd(
        nc, in_maps, core_ids=list(range(NC)), trace=trace
    )
    outs = []
    for c in range(NC):
        o = results.results[c]["out"]
        outs.append(o[data[(c, 0)]["outperm"]])  # slot0 order -> node order
    full = np.concatenate(outs, axis=0).astype(np.float32)
    return full, results


def kernel(h, src, dst, W_fc, a_attn):
    full, _ = run(h, src, dst, W_fc, a_attn)
    return full
